# revision 1
# baseline (speedup 1.0000x reference)
"""Multi-headed self-attention (S=2048, D=1024, H=16) on 8 trn2 NeuronCores.

Sharding: tensor-parallel over heads (2 heads/core). Each core computes
qkv for its heads in transposed layout (so the softmaxed probabilities
feed the ctx matmul without a transpose), uses a no-max base-2 softmax
(2^s / sum 2^s == 2^(s-max) / sum 2^(s-max), with the denominator from
a fused ones-column in v and broadcast via a K=1 outer-product matmul),
then four small AllToAlls (one per head x s-half, all but the last
overlapped with compute) reshard from head-split to sequence-split for
the output projection. Host reassembles the 8 cores' two row-strips.

Self-contained: hardcodes all shapes; host-side prep is limited to
transpose / dtype-cast / slicing of the inputs.
"""

import sys

import numpy as np

if "/opt/trn_rl_repo" not in sys.path:
    sys.path.insert(0, "/opt/trn_rl_repo")

S, D, A, H = 2048, 1024, 1024, 16
NCORES = 8
HPC = H // NCORES            # heads per core = 2
HD = A // H                  # head dim = 64
E = HPC * HD                 # local ctx rows = 128
ND = D // 128                # d tiles = 8
NT = S // 128                # t tiles = 16
LN2 = 0.6931471805599453
EXP_SCALE = LN2 * (HD ** -0.5)   # p = 2^(score/8) = exp(score * ln2/8)

# attention s-chunking == ReduceScatter chunking
CH = 1024
NCH = S // CH
RSS = CH // NCORES           # rows per core per RS chunk = 128
SS = S // NCORES             # seq slice per core for proj = 256

_CACHE = {}


def _build(enable_asserts=False, debug_taps=False):
    import concourse.bass as bass
    import concourse.tile as tile
    import concourse.mybir as mybir
    from concourse import bacc
    from concourse.masks import make_identity

    f16 = mybir.dt.float16
    f32 = mybir.dt.float32

    nc = bacc.Bacc(
        "TRN2",
        target_bir_lowering=False,
        debug=False,
        enable_asserts=enable_asserts,
        num_devices=NCORES,
    )

    xT = nc.dram_tensor("xT", [D, S], f16, kind="ExternalInput").ap()
    wqT = nc.dram_tensor("wqT", [D, E], f16, kind="ExternalInput").ap()
    wkT = nc.dram_tensor("wkT", [D, E], f16, kind="ExternalInput").ap()
    wvT = nc.dram_tensor("wvT", [D, E], f16, kind="ExternalInput").ap()
    woT = nc.dram_tensor("woT", [A, D], f16, kind="ExternalInput").ap()
    out = nc.dram_tensor("out", [NCH, 128, D], f16, kind="ExternalOutput").ap()
    taps = None
    if debug_taps:
        taps = {
            name: nc.dram_tensor(name, shape, dt, kind="ExternalOutput").ap()
            for name, shape, dt in [
                ("dbg_qT", [128, S], f16),
                ("dbg_kT", [128, S], f16),
                ("dbg_vp", [128, NT * 2 * (HD + 1)], f16),
                ("dbg_pt", [128, CH], f16),
                ("dbg_ctxn0", [HD, S], f16),
                ("dbg_ctxn1", [HD, S], f16),
                ("dbg_outp", [128, D], f16),
            ]
        }

    with tile.TileContext(nc) as tc:
        _body(tc, xT, wqT, wkT, wvT, woT, out, mybir, bass, make_identity, taps)

    nc.compile()
    return nc


def _body(tc, xT, wqT, wkT, wvT, woT, out, mybir, bass, make_identity, taps=None):
    from contextlib import ExitStack

    nc = tc.nc
    f16 = mybir.dt.float16
    f32 = mybir.dt.float32
    Exp = mybir.ActivationFunctionType.Exp

    ctx_stack = ExitStack()
    # ---- persistent SBUF tensors (one bufs=1 pool, distinct tags) ----
    persist = ctx_stack.enter_context(tc.tile_pool(name="persist", bufs=1))

    def ptile(shape, dtype, name):
        return persist.tile(shape, dtype, tag=name, name=name)

    xt_sb = ptile([128, ND, S], f16, "xt_sb")        # x.T, d-tile major
    wq_sb = ptile([128, ND, E], f16, "wq_sb")
    wk_sb = ptile([128, ND, E], f16, "wk_sb")
    wv_sb = ptile([128, ND, E], f16, "wv_sb")
    wo_sb = ptile([128, ND, D], f16, "wo_sb")
    qT_sb = ptile([128, S], f16, "qT_sb")            # [2*hd, s]
    kT_sb = ptile([128, S], f16, "kT_sb")
    vT_sb = ptile([128, S], f16, "vT_sb")
    # v' per t-tile: [v_h0 | ones | v_h1 | ones] -> cols [0:65] and [65:130]
    vp_sb = ptile([128, NT, 2 * (HD + 1)], f16, "vp_sb")
    ident_sb = ptile([128, 128], f16, "ident_sb")
    ones_sb = ptile([HD + 1, HD], f16, "ones_sb")
    # normalized ctx.T per head (base partition 0 each)
    ctxn_h = [ptile([HD, S], f16, f"ctxn_h{h}") for h in range(HPC)]
    ctxf_sb = [
        ptile([128, NCORES, 128], f16, f"ctxf_sb{ci}") for ci in range(NCH)
    ]
    acc_sb = ptile([128, SS // 128, D], f32, "acc_sb")

    make_identity(nc, ident_sb[:])
    nc.vector.memset(ones_sb[:], 1.0)

    # ---- load inputs (batched; xT per d-tile for finer overlap) ----
    for dt_ in range(ND):
        nc.sync.dma_start(wk_sb[:, dt_, :], wkT[dt_ * 128:(dt_ + 1) * 128, :])
        nc.sync.dma_start(wq_sb[:, dt_, :], wqT[dt_ * 128:(dt_ + 1) * 128, :])
        nc.sync.dma_start(wv_sb[:, dt_, :], wvT[dt_ * 128:(dt_ + 1) * 128, :])
        for qq in range(4):
            nc.sync.dma_start(
                xt_sb[:, dt_, qq * 512:(qq + 1) * 512],
                xT[dt_ * 128:(dt_ + 1) * 128, qq * 512:(qq + 1) * 512],
            )
    nc.sync.dma_start(wo_sb[:], woT.rearrange("(a p) d -> p a d", p=128))

    # ---- qkv.T = w.T^T @ x.T : d-tile outer so each weight LDW feeds 4 MMs
    with tc.tile_pool(name="qkv_ps", bufs=2, space="PSUM") as qkv_ps:
        for w_sb, dst in ((wk_sb, kT_sb), (wq_sb, qT_sb), (wv_sb, vT_sb)):
            pss = [
                qkv_ps.tile([128, 512], f32, tag=f"qkv{i}", name=f"qkv{i}")
                for i in range(4)
            ]
            for dt_ in range(ND):
                for sc in range(4):
                    nc.tensor.matmul(
                        pss[sc][:],
                        lhsT=w_sb[:, dt_, :],
                        rhs=xt_sb[:, dt_, sc * 512:(sc + 1) * 512],
                        start=(dt_ == 0),
                        stop=(dt_ == ND - 1),
                    )
            for sc in range(4):
                nc.vector.tensor_copy(dst[:, sc * 512:(sc + 1) * 512], pss[sc][:])

    # ---- v' = v.T transposed back per t-tile, plus ones columns ----
    with tc.tile_pool(name="tr_ps", bufs=3, space="PSUM") as tr_ps:
        for tt in range(NT):
            tp = tr_ps.tile([128, 128], f16, tag="tr")
            nc.tensor.transpose(
                tp[:], vT_sb[:, tt * 128:(tt + 1) * 128], ident_sb[:]
            )
            nc.vector.tensor_copy(vp_sb[:, tt, 0:HD], tp[:, 0:HD])
            nc.vector.tensor_copy(
                vp_sb[:, tt, HD + 1:2 * HD + 1], tp[:, HD:2 * HD]
            )
        nc.vector.memset(vp_sb[:, :, HD:HD + 1], 1.0)
        nc.vector.memset(vp_sb[:, :, 2 * HD + 1:2 * HD + 2], 1.0)

    if taps is not None:
        nc.sync.dma_start(taps["dbg_qT"][:], qT_sb[:])
        nc.sync.dma_start(taps["dbg_kT"][:], kT_sb[:])
        nc.sync.dma_start(taps["dbg_vp"][:], vp_sb[:].rearrange("p a b -> p (a b)"))

    # ---- attention + per-head AllToAll ----
    dram = ctx_stack.enter_context(tc.tile_pool(name="dram", bufs=1, space="DRAM"))
    a2a_in = [
        [
            dram.tile([NCORES, HD, 128], f16, name=f"a2a_in{h}_{ci}")
            for ci in range(NCH)
        ]
        for h in range(HPC)
    ]
    a2a_out = [
        [
            dram.tile([NCORES, HD, 128], f16, name=f"a2a_out{h}_{ci}")
            for ci in range(NCH)
        ]
        for h in range(HPC)
    ]

    with (
        tc.tile_pool(name="sc_ps", bufs=2, space="PSUM") as sc_ps,
        tc.tile_pool(name="ctx_ps", bufs=1, space="PSUM") as ctx_ps,
        tc.tile_pool(name="bc_ps", bufs=2, space="PSUM") as bc_ps,
        tc.tile_pool(name="pt_pool", bufs=4) as pt_pool,
        tc.tile_pool(name="bc_pool", bufs=2) as bc_pool,
        tc.tile_pool(name="den_pool", bufs=2) as den_pool,
    ):
        for h in range(HPC):
            hb = h * HD      # head base partition
            for ci in range(NCH):
                ctx = ctx_ps.tile([HD + 1, CH], f32, tag="ctx", name="ctx")
                for tt in range(NT):
                    sc = sc_ps.tile([128, CH], f32, tag="sc", name="sc")
                    for nn in range(CH // 512):
                        nc.tensor.matmul(
                            sc[:, nn * 512:(nn + 1) * 512],
                            lhsT=kT_sb[hb:hb + HD, tt * 128:(tt + 1) * 128],
                            rhs=qT_sb[hb:hb + HD,
                                      ci * CH + nn * 512:ci * CH + (nn + 1) * 512],
                            start=True,
                            stop=True,
                            tile_position=(hb, 0),
                        )
                    pt = pt_pool.tile([128, CH], f16, tag="pt")
                    nc.scalar.activation(pt[:], sc[:], Exp, scale=EXP_SCALE)
                    if taps is not None and h == 0 and ci == 0 and tt == 0:
                        nc.sync.dma_start(taps["dbg_pt"][:], pt[:])
                    for nn in range(CH // 512):
                        nc.tensor.matmul(
                            ctx[:, nn * 512:(nn + 1) * 512],
                            lhsT=vp_sb[:, tt, h * (HD + 1):(h + 1) * (HD + 1)],
                            rhs=pt[:, nn * 512:(nn + 1) * 512],
                            start=(tt == 0),
                            stop=(tt == NT - 1),
                        )
                # softmax denominator: row HD of ctx psum; normalize and
                # bounce out per 256-wide sub-chunk (= one rank block) so
                # the chain pipelines and nothing big sits on the tail
                for sub in range(CH // SS):
                    r = ci * (CH // SS) + sub
                    s0 = sub * SS
                    den = den_pool.tile([HD + 1, SS], f16, tag="den", name="den")
                    nc.vector.tensor_copy(
                        den[HD:HD + 1, :], ctx[HD:HD + 1, s0:s0 + SS]
                    )
                    # broadcast across partitions via K=1 outer product
                    bcp = bc_ps.tile([HD, SS], f32, tag="bcp", name="bcp")
                    nc.tensor.matmul(
                        bcp[:],
                        lhsT=ones_sb[HD:HD + 1, :],
                        rhs=den[HD:HD + 1, :],
                        start=True,
                        stop=True,
                        tile_position=(HD, 0),
                    )
                    rbc = bc_pool.tile([HD, SS], f32, tag="rbc", name="rbc")
                    nc.vector.reciprocal_approx_fast(rbc[:], bcp[:])
                    nc.vector.tensor_mul(
                        ctxn_h[h][:, r * SS:(r + 1) * SS],
                        ctx[0:HD, s0:s0 + SS],
                        rbc[:],
                    )
                    for half in range(2):
                        blk = 2 * sub + half
                        nc.scalar.dma_start(
                            a2a_in[h][ci][blk],
                            ctxn_h[h][:, ci * CH + blk * 128:
                                       ci * CH + (blk + 1) * 128],
                        )
                nc.gpsimd.collective_compute(
                    "AllToAll",
                    mybir.AluOpType.bypass,
                    replica_groups=[list(range(NCORES))],
                    ins=[a2a_in[h][ci].opt()],
                    outs=[a2a_out[h][ci].opt()],
                )
                for r in range(NCORES):
                    nc.gpsimd.dma_start(
                        ctxf_sb[ci][h * HD:(h + 1) * HD, r, :],
                        a2a_out[h][ci][r],
                    )

        if taps is not None:
            nc.sync.dma_start(taps["dbg_ctxn0"][:], ctxn_h[0][:])
            nc.sync.dma_start(taps["dbg_ctxn1"][:], ctxn_h[1][:])

        # ---- reload: ctxf[:, k, :] rows 0:64 = head-even block k, 64:128 odd ----
        # proj is K-split by head parity: the even-head half (phase A) only
        # needs a2a_out[0], so it runs during the second AllToAll; phase B
        # accumulates the odd-head half on top via SBUF.

        with tc.tile_pool(name="out_pool", bufs=2) as out_pool:
            for ci in range(NCH):
                ob = out_pool.tile([128, D], f16, tag="ob", name="ob")
                for nn in range(2):
                    ps = sc_ps.tile([128, 512], f32, tag="sc", name="proj")
                    for kt in range(ND):
                        nc.tensor.matmul(
                            ps[:],
                            lhsT=ctxf_sb[ci][:, kt, :],
                            rhs=wo_sb[:, kt, nn * 512:(nn + 1) * 512],
                            start=(kt == 0),
                            stop=(kt == ND - 1),
                        )
                    nc.vector.tensor_copy(ob[:, nn * 512:(nn + 1) * 512], ps[:])
                nc.scalar.dma_start(out[ci], ob[:])
                if taps is not None and ci == 0:
                    nc.sync.dma_start(taps["dbg_outp"][:], ob[:])

    ctx_stack.close()


def get_nc(enable_asserts=False, debug_taps=False):
    key = ("nc", enable_asserts, debug_taps)
    if key not in _CACHE:
        _CACHE[key] = _build(enable_asserts, debug_taps)
    return _CACHE[key]


def make_in_maps(x, w_in, w_out):
    x = np.asarray(x, dtype=np.float32)
    w_in = np.asarray(w_in, dtype=np.float32)
    w_out = np.asarray(w_out, dtype=np.float32)
    xT = np.ascontiguousarray(x.T).astype(np.float16)
    w_outT = w_out.T.astype(np.float16)          # [A(e), D]
    in_maps = []
    for c in range(NCORES):
        r0 = c * E
        wq = np.ascontiguousarray(w_in[r0:r0 + E].T).astype(np.float16)
        wk = np.ascontiguousarray(w_in[A + r0:A + r0 + E].T).astype(np.float16)
        wv = np.ascontiguousarray(
            w_in[2 * A + r0:2 * A + r0 + E].T
        ).astype(np.float16)
        in_maps.append(
            {"xT": xT, "wqT": wq, "wkT": wk, "wvT": wv, "woT": w_outT}
        )
    return in_maps


def assemble_out(results):
    """results[c]["out"] is [NCH, 128, D] fp16; strip ci = out rows
    [ci*CH + c*128 : +128]."""
    full = np.empty((S, D), dtype=np.float32)
    for c in range(NCORES):
        o = results[c]["out"]
        for ci in range(NCH):
            r0 = ci * CH + c * 128
            full[r0:r0 + 128] = o[ci].astype(np.float32)
    return full


def kernel(x, w_in, w_out, tgt_len=None, **kwargs):
    from concourse.bass_utils import run_bass_kernel_spmd

    nc = get_nc()
    in_maps = make_in_maps(x, w_in, w_out)
    res = run_bass_kernel_spmd(nc, in_maps, core_ids=list(range(NCORES)))
    return assemble_out(res.results)



# revision 2
# speedup vs baseline: 1.0274x; 1.0274x over previous
"""Multi-headed self-attention (S=2048, D=1024, H=16) on 8 trn2 NeuronCores.

v3: ACT-engine-centric design with PE/collective warmup. The exp of
S*S*2-head scores (8.4M elems/core) is the hard floor (~72us on the ACT
engine); everything else hides under it:

- tensor-parallel over heads (2 heads/core), seq chunks of CH=512 queries.
- per (chunk, key-tile): ONE [128,1024] fp32 PSUM score tile holding both
  heads side by side (row-packed concurrent K=64 matmuls via tile_position),
  ONE 1024-wide exp activation, then 2 ctx matmuls (K=128, M=65 with a
  fused ones-column producing the softmax denominator in row 64).
- t=0: ~40 identity matmuls flip the PE HAM clock-gate to 8/8 while inputs
  stream in; a 1KB dummy AllToAll absorbs the first-collective barrier and
  ncfw warmup (~45us) during the prologue.
- k/q/v computed in 512-col chunks interleaved into the first chunk's ACT
  slack; each chunk's ctx matmuls run 2-per-slot in its second half;
  normalization (denominator bcast via K=1 matmul + reciprocal) is emitted
  into the next chunk's first slots; one 128KB AllToAll per chunk.
- output projection per chunk-PAIR (M=128): pair 0 inside chunk 3's slack,
  pair 1 in the tail under the last AllToAll's latency.

Self-contained: hardcodes all shapes; host-side prep is limited to
transpose / dtype-cast / slicing of the inputs.
"""

import sys

import numpy as np

if "/opt/trn_rl_repo" not in sys.path:
    sys.path.insert(0, "/opt/trn_rl_repo")

S, D, A, H = 2048, 1024, 1024, 16
NCORES = 8
HPC = H // NCORES            # heads per core = 2
HD = A // H                  # head dim = 64
E = HPC * HD                 # local qkv rows = 128
ND = D // 128                # contraction d tiles = 8
NT = S // 128                # key tiles = 16
LN2 = 0.6931471805599453
EXP_SCALE = LN2 * (HD ** -0.5)   # p = 2^(score/8) = exp(score * ln2/8)

CH = 512                     # queries per attention chunk
NCH = S // CH                # 4 chunks
RQ = CH // NCORES            # queries per rank per chunk = 64
NP = NCH // 2                # proj pairs (M=128 rows each)

_CACHE = {}


def _build(enable_asserts=False):
    import concourse.bass as bass
    import concourse.tile as tile
    import concourse.mybir as mybir
    from concourse import bacc
    from concourse.masks import make_identity

    f16 = mybir.dt.float16

    nc = bacc.Bacc(
        "TRN2",
        target_bir_lowering=False,
        debug=False,
        enable_asserts=enable_asserts,
        num_devices=NCORES,
    )

    # host-pre-arranged for fully contiguous DMA on both sides:
    # xTb[cb][p, dt, s512], w*b[p, dt, e], wob[p, a, d]
    xTb = nc.dram_tensor("xTb", [4, 128, ND, 512], f16, kind="ExternalInput").ap()
    wqb = nc.dram_tensor("wqb", [128, ND, E], f16, kind="ExternalInput").ap()
    wkb = nc.dram_tensor("wkb", [128, ND, E], f16, kind="ExternalInput").ap()
    wvb = nc.dram_tensor("wvb", [128, ND, E], f16, kind="ExternalInput").ap()
    wob = nc.dram_tensor("wob", [128, ND, D], f16, kind="ExternalInput").ap()
    # out[p] rows: 0:64 -> global rows p*1024 + c*64, 64:128 -> p*1024+512+c*64
    out = nc.dram_tensor("out", [NP, 128, D], f16, kind="ExternalOutput").ap()

    with tile.TileContext(nc) as tc:
        _body(tc, xTb, wqb, wkb, wvb, wob, out, mybir, bass, make_identity)

    nc.compile()
    return nc


def _body(tc, xTb, wqb, wkb, wvb, wob, out, mybir, bass, make_identity):
    from contextlib import ExitStack

    nc = tc.nc
    f16 = mybir.dt.float16
    f32 = mybir.dt.float32
    Exp = mybir.ActivationFunctionType.Exp

    es = ExitStack()
    persist = es.enter_context(tc.tile_pool(name="persist", bufs=1))

    def ptile(shape, dtype, name):
        return persist.tile(shape, dtype, tag=name, name=name)

    xt_sb = ptile([128, ND, S], f16, "xt_sb")        # x.T, d-tile major
    wq_sb = ptile([128, ND, E], f16, "wq_sb")
    wk_sb = ptile([128, ND, E], f16, "wk_sb")
    wv_sb = ptile([128, ND, E], f16, "wv_sb")
    wo_sb = ptile([128, ND, D], f16, "wo_sb")
    qT_sb = ptile([128, S], f16, "qT_sb")            # [2*hd, s]
    kT_sb = ptile([128, S], f16, "kT_sb")
    vT_sb = ptile([128, S], f16, "vT_sb")
    # v' per key-tile: [v_h0 | ones | v_h1 | ones] -> lhsT slices [0:65],[65:130]
    vp_sb = ptile([128, NT, 2 * (HD + 1)], f16, "vp_sb")
    ident_sb = ptile([128, 128], f16, "ident_sb")
    ones_sb = ptile([HD + 1, HD], f16, "ones_sb")
    ctxf_sb = [ptile([128, ND, 128], f16, f"ctxf{p}") for p in range(NP)]

    warm_sb = ptile([1, NCORES * 64], f16, "warm_sb")
    make_identity(nc, ident_sb[:])
    nc.vector.memset(warm_sb[:], 0.0)
    nc.vector.memset(ones_sb[:], 1.0)
    nc.vector.memset(vp_sb[:, :, HD:HD + 1], 1.0)
    nc.vector.memset(vp_sb[:, :, 2 * HD + 1:2 * HD + 2], 1.0)

    dram = es.enter_context(tc.tile_pool(name="dram", bufs=1, space="DRAM"))
    a2a_inA = dram.tile([NCORES, 128, 2 * RQ], f16, name="a2a_inA")
    a2a_outA = dram.tile([NCORES, 128, 2 * RQ], f16, name="a2a_outA")
    a2a_inB = [dram.tile([NCORES, 128, RQ], f16, name=f"a2a_inB{i}")
               for i in range(2)]
    a2a_outB = [dram.tile([NCORES, 128, RQ], f16, name=f"a2a_outB{i}")
                for i in range(2)]
    warm_in = dram.tile([NCORES, 1, 64], f16, name="warm_in")
    warm_out = dram.tile([NCORES, 1, 64], f16, name="warm_out")

    # warm the collective path first: absorbs the first-collective barrier
    # and ncfw warmup (~30-50us) while the qkv prologue runs
    nc.sync.dma_start(warm_in.rearrange("r p q -> p (r q)"), warm_sb[:])
    nc.gpsimd.collective_compute(
        "AllToAll",
        mybir.AluOpType.bypass,
        replica_groups=[list(range(NCORES))],
        ins=[warm_in.opt()],
        outs=[warm_out.opt()],
    )

    # ---- input loads: one rearranged DMA per weight (2KB/partition lines),
    # x in four 1MB column blocks so q/k chunk 0 starts as early as possible
    def x_chunk_dma(cb):
        nc.sync.dma_start(xt_sb[:, :, cb * 512:(cb + 1) * 512], xTb[cb])

    nc.sync.dma_start(wq_sb[:], wqb)
    nc.sync.dma_start(wk_sb[:], wkb)
    x_chunk_dma(0)
    nc.sync.dma_start(wv_sb[:], wvb)
    for cb in range(1, 4):
        x_chunk_dma(cb)

    sc_ps = es.enter_context(tc.tile_pool(name="sc_ps", bufs=2, space="PSUM"))
    pt_pool = es.enter_context(tc.tile_pool(name="pt_pool", bufs=16))
    cn_pool = es.enter_context(tc.tile_pool(name="cn_pool", bufs=2))
    den_pool = es.enter_context(tc.tile_pool(name="den_pool", bufs=2))
    rbc_pool = es.enter_context(tc.tile_pool(name="rbc_pool", bufs=2))
    ob_pool = es.enter_context(tc.tile_pool(name="ob_pool", bufs=2))

    qkv_cnt = [0]

    def qkv_chunk(dst, w_sb, cb, qkv_ps):
        """dst[:, cb*512:(cb+1)*512] accumulated over all 8 d-tiles."""
        tag = f"qkv{qkv_cnt[0] % 2}"
        qkv_cnt[0] += 1
        ps = qkv_ps.tile([128, 512], f32, tag=tag, name=tag)
        for dt in range(ND):
            nc.tensor.matmul(
                ps[:],
                lhsT=w_sb[:, dt, :],
                rhs=xt_sb[:, dt, cb * 512:(cb + 1) * 512],
                start=(dt == 0),
                stop=(dt == ND - 1),
            )
        nc.vector.tensor_copy(dst[:, cb * 512:(cb + 1) * 512], ps[:])

    def sc_exp(ci, tt):
        """scores for both heads of key-tile tt -> exp -> pt [128, 2*CH]."""
        sc = sc_ps.tile([128, 2 * CH], f32, tag="sc", name="sc")
        for h in range(2):
            nc.tensor.matmul(
                sc[:, h * CH:(h + 1) * CH],
                lhsT=kT_sb[h * HD:(h + 1) * HD, tt * 128:(tt + 1) * 128],
                rhs=qT_sb[h * HD:(h + 1) * HD, ci * CH:(ci + 1) * CH],
                start=True,
                stop=True,
                tile_position=(h * HD, 0),
            )
        pt = pt_pool.tile([128, 2 * CH], f16, tag="pt", name="pt")
        nc.scalar.activation(pt[:], sc[:], Exp, scale=EXP_SCALE)
        return pt

    def ctx_mm(ctx_ts, tt, pt):
        for h in range(2):
            nc.tensor.matmul(
                ctx_ts[h][:],
                lhsT=vp_sb[:, tt, h * (HD + 1):(h + 1) * (HD + 1)],
                rhs=pt[:, h * CH:(h + 1) * CH],
                start=(tt == 0),
                stop=(tt == NT - 1),
            )

    def norm_a2a(ci, ctx_ts, bc_ps, first=False):
        # denominator row (partition HD of ctx psum) -> sbuf, K=1 matmul
        # broadcast across partitions 0:HD, reciprocal, scale, ship.
        dens = []
        for h in range(2):
            den = den_pool.tile([HD + 1, CH], f16, tag="den", name="den")
            nc.vector.tensor_copy(den[HD:HD + 1, :], ctx_ts[h][HD:HD + 1, :])
            dens.append(den)
        for h in range(2):
            # bc shares the proj PSUM bank (single-buffered): keep the
            # bcast->recip chain strictly per-head so h1's matmul can't
            # clobber h0's bcast before its reciprocal reads it
            bcp = bc_ps.tile([128, CH], f32, tag="pj", name="bcp")
            nc.tensor.matmul(
                bcp[0:HD, :],
                lhsT=ones_sb[HD:HD + 1, :],
                rhs=dens[h][HD:HD + 1, :],
                start=True,
                stop=True,
                tile_position=(HD, 0),
            )
            rb = rbc_pool.tile([HD, CH], f32, tag="rbc", name="rbc")
            nc.vector.reciprocal_approx_fast(rb[:], bcp[0:HD, :])
            cnh = cn_pool.tile([HD, CH], f16, tag=f"cn{h}", name=f"cn{h}")
            nc.vector.tensor_mul(cnh[:], ctx_ts[h][0:HD, :], rb[:])
            if ci <= 1:
                dst = a2a_inA[:, h * HD:(h + 1) * HD,
                              ci * RQ:(ci + 1) * RQ]
            else:
                dst = a2a_inB[ci - 2][:, h * HD:(h + 1) * HD, :]
            nc.sync.dma_start(
                dst.rearrange("r p q -> p r q"),
                cnh[:].rearrange("p (r q) -> p r q", r=NCORES),
            )
        if first:
            # w_out load deferred here so it never contends with x loads
            nc.sync.dma_start(wo_sb[:], wob)

        def a2a(ins, outs):
            nc.gpsimd.collective_compute(
                "AllToAll",
                mybir.AluOpType.bypass,
                replica_groups=[list(range(NCORES))],
                ins=[ins.opt()],
                outs=[outs.opt()],
            )

        if ci == 1:
            # pair A complete: 256KB AllToAll, pull all of ctxf[0]
            a2a(a2a_inA, a2a_outA)
            nc.gpsimd.dma_start(
                ctxf_sb[0][:], a2a_outA.rearrange("r p q -> p r q")
            )
        elif ci >= 2:
            # chunk-sized AllToAll right away: B2 hides under chunk 3's
            # compute, B3 under the tail half-projection
            a2a(a2a_inB[ci - 2], a2a_outB[ci - 2])
            nc.gpsimd.dma_start(
                ctxf_sb[1][:, :, (ci - 2) * RQ:(ci - 1) * RQ],
                a2a_outB[ci - 2].rearrange("r p q -> p r q"),
            )

    def proj_steps(p, state, proj_ps, steps):
        """Emit `steps` increments of proj pair p. 18 steps total: 8 kt
        matmuls + copy for nn=0, then the same for nn=1, then the output
        DMA. One single-bank PSUM accumulator, reused across the halves."""
        for _ in range(steps):
            i = state.get("i", 0)
            if i >= 18:
                return
            state["i"] = i + 1
            if i == 0:
                state["ob"] = ob_pool.tile([128, D], f16, tag="ob", name="ob")
            nn, j = divmod(i, 9)
            if j == 0:
                state["pp"] = proj_ps.tile(
                    [128, 512], f32, tag="pj", name="pj"
                )
            if j < 8:
                nc.tensor.matmul(
                    state["pp"][:],
                    lhsT=ctxf_sb[p][:, j, :],
                    rhs=wo_sb[:, j, nn * 512:(nn + 1) * 512],
                    start=(j == 0),
                    stop=(j == ND - 1),
                )
            else:
                nc.vector.tensor_copy(
                    state["ob"][:, nn * 512:(nn + 1) * 512], state["pp"][:]
                )
                nc.sync.dma_start(
                    out[p][:, nn * 512:(nn + 1) * 512],
                    state["ob"][:, nn * 512:(nn + 1) * 512],
                )

    # ---- phase 1: PE warmup, all qkv chunks + chunk-0 scores, transposes
    pts = {}
    with (
        tc.tile_pool(name="qkv_ps", bufs=1, space="PSUM") as qkv_ps,
        tc.tile_pool(name="tr_ps", bufs=2, space="PSUM") as tr_ps,
    ):
        # ~40 back-to-back identity matmuls flip HAM to 8/8 (~3.4us) while
        # the input DMAs stream in
        warm = qkv_ps.tile([128, 512], f32, tag="qkv0", name="warm")
        for _ in range(48):
            nc.tensor.matmul(
                warm[:, 0:128], lhsT=ident_sb[:, 0:128],
                rhs=ident_sb[:, 0:128], start=True, stop=True,
            )
        qkv_chunk(qT_sb, wq_sb, 0, qkv_ps)
        qkv_chunk(kT_sb, wk_sb, 0, qkv_ps)
        pts[0] = sc_exp(0, 0)
        pts[1] = sc_exp(0, 1)
        qkv_chunk(kT_sb, wk_sb, 1, qkv_ps)
        pts[2] = sc_exp(0, 2)
        pts[3] = sc_exp(0, 3)
        qkv_chunk(kT_sb, wk_sb, 2, qkv_ps)
        pts[4] = sc_exp(0, 4)
        qkv_chunk(kT_sb, wk_sb, 3, qkv_ps)
        pts[5] = sc_exp(0, 5)
        def tr_block(t0, t1):
            for tt in range(t0, t1):
                tp = tr_ps.tile([128, 128], f16, tag="tr", name="tr")
                nc.tensor.transpose(
                    tp[:], vT_sb[:, tt * 128:(tt + 1) * 128], ident_sb[:]
                )
                nc.vector.tensor_copy(vp_sb[:, tt, 0:HD], tp[:, 0:HD])
                nc.vector.tensor_copy(
                    vp_sb[:, tt, HD + 1:2 * HD + 1], tp[:, HD:2 * HD]
                )

        qkv_chunk(vT_sb, wv_sb, 0, qkv_ps)
        pts[6] = sc_exp(0, 6)
        tr_block(0, 4)
        qkv_chunk(vT_sb, wv_sb, 1, qkv_ps)
        pts[7] = sc_exp(0, 7)
        tr_block(4, 8)
        qkv_chunk(vT_sb, wv_sb, 2, qkv_ps)
        pts[8] = sc_exp(0, 8)
        tr_block(8, 12)
        qkv_chunk(vT_sb, wv_sb, 3, qkv_ps)
        pts[9] = sc_exp(0, 9)
        tr_block(12, 16)
        qkv_chunk(qT_sb, wq_sb, 1, qkv_ps)

    # ---- phase 2: psum = sc 4 + ctx 2 + (bc/proj shared) 1 + q23 1 banks --
    with (
        tc.tile_pool(name="ctx_ps", bufs=1, space="PSUM") as ctx_ps,
        tc.tile_pool(name="proj_ps", bufs=1, space="PSUM") as proj_ps,
        tc.tile_pool(name="q23_ps", bufs=1, space="PSUM") as q23_ps,
    ):
        def ctx_tiles():
            return [
                ctx_ps.tile([HD + 1, CH], f32, tag=f"ctx{h}", name=f"ctx{h}")
                for h in range(2)
            ]

        # chunk 0 remaining: sc 10..15 with ctx t=0..11 two-per-slot;
        # every chunk leaves its last ctx t's as leftovers for the next
        # chunk's first slots so the exp stream never waits at boundaries
        cts0 = ctx_tiles()
        for tt in range(10, NT):
            pts[tt] = sc_exp(0, tt)
            for t in (2 * (tt - 10), 2 * (tt - 10) + 1):
                ctx_mm(cts0, t, pts[t])

        prev_cts, prev_pts = cts0, pts
        prev_left = [12, 13, 14, 15]
        pstate = [dict(), dict()]
        for ci in range(1, NCH):
            cts = ctx_tiles()
            cpts = {}
            fillers = []
            if ci in (1, 2):
                # q chunk for the next chunk's queries, one matmul per slot
                cb = ci + 1
                qp = q23_ps.tile([128, 512], f32, tag="q23", name="q23")

                def _qmm(dt, qp=qp, cb=cb):
                    nc.tensor.matmul(
                        qp[:],
                        lhsT=wq_sb[:, dt, :],
                        rhs=xt_sb[:, dt, cb * 512:(cb + 1) * 512],
                        start=(dt == 0),
                        stop=(dt == ND - 1),
                    )

                def _qcopy(qp=qp, cb=cb):
                    nc.vector.tensor_copy(
                        qT_sb[:, cb * 512:(cb + 1) * 512], qp[:]
                    )

                fillers = [lambda dt=dt: _qmm(dt) for dt in range(ND)]
                fillers.append(_qcopy)
            for tt in range(NT):
                cpts[tt] = sc_exp(ci, tt)
                if tt <= 1 and prev_left:
                    k = len(prev_left) // 2
                    for t in (prev_left[:k] if tt == 0 else prev_left[k:]):
                        ctx_mm(prev_cts, t, prev_pts[t])
                if fillers and tt >= 1:
                    fillers.pop(0)()
                if tt == 3:
                    norm_a2a(ci - 1, prev_cts, proj_ps, first=(ci == 1))
                if ci == 3 and 4 <= tt < 13:
                    proj_steps(0, pstate[0], proj_ps, 2)
                if 8 <= tt < 15:
                    for t in (2 * (tt - 8), 2 * (tt - 8) + 1):
                        ctx_mm(cts, t, cpts[t])
            prev_cts, prev_pts = cts, cpts
            prev_left = [14, 15]
        for t in (14, 15):
            ctx_mm(prev_cts, t, prev_pts[t])
        norm_a2a(NCH - 1, prev_cts, proj_ps)
        # tail: pair-1 projection in M=64 halves — the ci2 half's data
        # (B2) landed during chunk 3, so it runs during B3's flight and
        # doubles as the PE keep-alive; the ci3 half follows B3
        proj_steps(0, pstate[0], proj_ps, 18)
        ob1 = ob_pool.tile([128, D], f16, tag="ob", name="ob1")
        for half in range(2):
            for nn in range(2):
                pp = proj_ps.tile([128, 512], f32, tag="pj", name="pjh")
                for kt in range(ND):
                    nc.tensor.matmul(
                        pp[half * HD:(half + 1) * HD, :],
                        lhsT=ctxf_sb[1][:, kt, half * HD:(half + 1) * HD],
                        rhs=wo_sb[:, kt, nn * 512:(nn + 1) * 512],
                        start=(kt == 0),
                        stop=(kt == ND - 1),
                        tile_position=(0, half * HD),
                    )
                nc.vector.tensor_copy(
                    ob1[half * HD:(half + 1) * HD, nn * 512:(nn + 1) * 512],
                    pp[half * HD:(half + 1) * HD, :],
                )
                nc.sync.dma_start(
                    out[1][half * HD:(half + 1) * HD,
                           nn * 512:(nn + 1) * 512],
                    ob1[half * HD:(half + 1) * HD, nn * 512:(nn + 1) * 512],
                )

    es.close()


def get_nc(enable_asserts=False):
    key = ("nc", enable_asserts)
    if key not in _CACHE:
        _CACHE[key] = _build(enable_asserts)
    return _CACHE[key]


def make_in_maps(x, w_in, w_out):
    x = np.asarray(x, dtype=np.float32)
    w_in = np.asarray(w_in, dtype=np.float32)
    w_out = np.asarray(w_out, dtype=np.float32)
    xT = x.T.astype(np.float16)                  # [D, S]
    # [4, 128, ND, 512]: cb-block -> partition -> d-tile -> seq
    xTb = np.ascontiguousarray(
        xT.reshape(ND, 128, 4, 512).transpose(2, 1, 0, 3)
    )
    w_outT = w_out.T.astype(np.float16)          # [A, D]
    wob = np.ascontiguousarray(
        w_outT.reshape(ND, 128, D).transpose(1, 0, 2)
    )

    def wslice(w):                               # [E, D] -> [128, ND, E]
        return np.ascontiguousarray(
            w.T.astype(np.float16).reshape(ND, 128, E).transpose(1, 0, 2)
        )

    in_maps = []
    for c in range(NCORES):
        r0 = c * E
        in_maps.append({
            "xTb": xTb,
            "wqb": wslice(w_in[r0:r0 + E]),
            "wkb": wslice(w_in[A + r0:A + r0 + E]),
            "wvb": wslice(w_in[2 * A + r0:2 * A + r0 + E]),
            "wob": wob,
        })
    return in_maps


def assemble_out(results):
    """results[c]["out"] is [NP, 128, D] fp16; pair p rows 0:64 are global
    rows p*1024 + c*64, rows 64:128 are p*1024 + 512 + c*64."""
    full = np.empty((S, D), dtype=np.float32)
    for c in range(NCORES):
        o = results[c]["out"]
        for p in range(NP):
            r0 = p * 2 * CH + c * RQ
            full[r0:r0 + RQ] = o[p, 0:RQ].astype(np.float32)
            r1 = p * 2 * CH + CH + c * RQ
            full[r1:r1 + RQ] = o[p, RQ:2 * RQ].astype(np.float32)
    return full


def kernel(x, w_in, w_out, tgt_len=None, **kwargs):
    from concourse.bass_utils import run_bass_kernel_spmd

    nc = get_nc()
    in_maps = make_in_maps(x, w_in, w_out)
    res = run_bass_kernel_spmd(nc, in_maps, core_ids=list(range(NCORES)))
    return assemble_out(res.results)


# revision 3
# speedup vs baseline: 1.0348x; 1.0072x over previous
"""Multi-headed self-attention (S=2048, D=1024, H=16) on 8 trn2 NeuronCores.

v3: ACT-engine-centric design with PE/collective warmup. The exp of
S*S*2-head scores (8.4M elems/core) is the hard floor (~72us on the ACT
engine); everything else hides under it:

- tensor-parallel over heads (2 heads/core), seq chunks of CH=512 queries.
- per (chunk, key-tile): ONE [128,1024] fp32 PSUM score tile holding both
  heads side by side (row-packed concurrent K=64 matmuls via tile_position),
  ONE 1024-wide exp activation, then 2 ctx matmuls (K=128, M=65 with a
  fused ones-column producing the softmax denominator in row 64).
- t=0: ~40 identity matmuls flip the PE HAM clock-gate to 8/8 while inputs
  stream in; a 1KB dummy AllToAll absorbs the first-collective barrier and
  ncfw warmup (~45us) during the prologue.
- k/q/v computed in 512-col chunks interleaved into the first chunk's ACT
  slack; each chunk's ctx matmuls run 2-per-slot in its second half;
  normalization (denominator bcast via K=1 matmul + reciprocal) is emitted
  into the next chunk's first slots; one 128KB AllToAll per chunk.
- output projection per chunk-PAIR (M=128): pair 0 inside chunk 3's slack,
  pair 1 in the tail under the last AllToAll's latency.

Self-contained: hardcodes all shapes; host-side prep is limited to
transpose / dtype-cast / slicing of the inputs.
"""

import sys

import numpy as np

if "/opt/trn_rl_repo" not in sys.path:
    sys.path.insert(0, "/opt/trn_rl_repo")

S, D, A, H = 2048, 1024, 1024, 16
NCORES = 8
HPC = H // NCORES            # heads per core = 2
HD = A // H                  # head dim = 64
E = HPC * HD                 # local qkv rows = 128
ND = D // 128                # contraction d tiles = 8
NT = S // 128                # key tiles = 16
LN2 = 0.6931471805599453
EXP_SCALE = LN2 * (HD ** -0.5)   # p = 2^(score/8) = exp(score * ln2/8)

CH = 512                     # queries per attention chunk
NCH = S // CH                # 4 chunks
RQ = CH // NCORES            # queries per rank per chunk = 64
NP = NCH // 2                # proj pairs (M=128 rows each)

_CACHE = {}


def _build(enable_asserts=False):
    import concourse.bass as bass
    import concourse.tile as tile
    import concourse.mybir as mybir
    from concourse import bacc
    from concourse.masks import make_identity

    f16 = mybir.dt.float16

    nc = bacc.Bacc(
        "TRN2",
        target_bir_lowering=False,
        debug=False,
        enable_asserts=enable_asserts,
        num_devices=NCORES,
    )

    # host-pre-arranged for fully contiguous DMA on both sides:
    # xTb[cb][p, dt, s512], w*b[p, dt, e], wob[p, a, d]
    xTb = nc.dram_tensor("xTb", [4, 128, ND, 512], f16, kind="ExternalInput").ap()
    wqb = nc.dram_tensor("wqb", [128, ND, E], f16, kind="ExternalInput").ap()
    wkb = nc.dram_tensor("wkb", [128, ND, E], f16, kind="ExternalInput").ap()
    wvb = nc.dram_tensor("wvb", [128, ND, E], f16, kind="ExternalInput").ap()
    wob = nc.dram_tensor("wob", [128, ND, D], f16, kind="ExternalInput").ap()
    # out[p] rows: 0:64 -> global rows p*1024 + c*64, 64:128 -> p*1024+512+c*64
    out = nc.dram_tensor("out", [NP, 128, D], f16, kind="ExternalOutput").ap()

    with tile.TileContext(nc) as tc:
        _body(tc, xTb, wqb, wkb, wvb, wob, out, mybir, bass, make_identity)

    nc.compile()
    return nc


def _body(tc, xTb, wqb, wkb, wvb, wob, out, mybir, bass, make_identity):
    from contextlib import ExitStack

    nc = tc.nc
    f16 = mybir.dt.float16
    f32 = mybir.dt.float32
    Exp = mybir.ActivationFunctionType.Exp

    es = ExitStack()
    persist = es.enter_context(tc.tile_pool(name="persist", bufs=1))

    def ptile(shape, dtype, name):
        return persist.tile(shape, dtype, tag=name, name=name)

    xt_sb = ptile([128, ND, S], f16, "xt_sb")        # x.T, d-tile major
    wq_sb = ptile([128, ND, E], f16, "wq_sb")
    wk_sb = ptile([128, ND, E], f16, "wk_sb")
    wv_sb = ptile([128, ND, E], f16, "wv_sb")
    wo_sb = ptile([128, ND, D], f16, "wo_sb")
    qT_sb = ptile([128, S], f16, "qT_sb")            # [2*hd, s]
    kT_sb = ptile([128, S], f16, "kT_sb")
    vT_sb = ptile([128, S], f16, "vT_sb")
    # v' per key-tile: [v_h0 | ones | v_h1 | ones] -> lhsT slices [0:65],[65:130]
    vp_sb = ptile([128, NT, 2 * (HD + 1)], f16, "vp_sb")
    ident_sb = ptile([128, 128], f16, "ident_sb")
    ones_sb = ptile([HD + 1, HD], f16, "ones_sb")
    ctxf_sb = [ptile([128, ND, 128], f16, f"ctxf{p}") for p in range(NP)]

    warm_sb = ptile([1, NCORES * 64], f16, "warm_sb")
    make_identity(nc, ident_sb[:])
    nc.vector.memset(warm_sb[:], 0.0)
    nc.vector.memset(ones_sb[:], 1.0)
    nc.vector.memset(vp_sb[:, :, HD:HD + 1], 1.0)
    nc.vector.memset(vp_sb[:, :, 2 * HD + 1:2 * HD + 2], 1.0)

    dram = es.enter_context(tc.tile_pool(name="dram", bufs=1, space="DRAM"))
    a2a_inA = dram.tile([NCORES, 128, 2 * RQ], f16, name="a2a_inA")
    a2a_outA = dram.tile([NCORES, 128, 2 * RQ], f16, name="a2a_outA")
    a2a_inB = [dram.tile([NCORES, 128, RQ], f16, name=f"a2a_inB{i}")
               for i in range(2)]
    a2a_outB = [dram.tile([NCORES, 128, RQ], f16, name=f"a2a_outB{i}")
                for i in range(2)]
    warm_in = dram.tile([NCORES, 1, 64], f16, name="warm_in")
    warm_out = dram.tile([NCORES, 1, 64], f16, name="warm_out")

    # warm the collective path first: absorbs the first-collective barrier
    # and ncfw warmup (~30-50us) while the qkv prologue runs
    nc.sync.dma_start(warm_in.rearrange("r p q -> p (r q)"), warm_sb[:])
    nc.gpsimd.collective_compute(
        "AllToAll",
        mybir.AluOpType.bypass,
        replica_groups=[list(range(NCORES))],
        ins=[warm_in.opt()],
        outs=[warm_out.opt()],
    )

    # ---- input loads: one rearranged DMA per weight (2KB/partition lines),
    # x in four 1MB column blocks so q/k chunk 0 starts as early as possible
    def x_chunk_dma(cb):
        nc.sync.dma_start(xt_sb[:, :, cb * 512:(cb + 1) * 512], xTb[cb])

    nc.sync.dma_start(wq_sb[:], wqb)
    nc.sync.dma_start(wk_sb[:], wkb)
    x_chunk_dma(0)
    nc.sync.dma_start(wv_sb[:], wvb)
    for cb in range(1, 4):
        x_chunk_dma(cb)

    sc_ps = es.enter_context(tc.tile_pool(name="sc_ps", bufs=2, space="PSUM"))
    pt_pool = es.enter_context(tc.tile_pool(name="pt_pool", bufs=16))
    cn_pool = es.enter_context(tc.tile_pool(name="cn_pool", bufs=2))
    den_pool = es.enter_context(tc.tile_pool(name="den_pool", bufs=2))
    rbc_pool = es.enter_context(tc.tile_pool(name="rbc_pool", bufs=2))
    ob_pool = es.enter_context(tc.tile_pool(name="ob_pool", bufs=2))

    qkv_cnt = [0]

    def qkv_chunk(dst, w_sb, cb, qkv_ps):
        """dst[:, cb*512:(cb+1)*512] accumulated over all 8 d-tiles."""
        tag = f"qkv{qkv_cnt[0] % 2}"
        qkv_cnt[0] += 1
        ps = qkv_ps.tile([128, 512], f32, tag=tag, name=tag)
        for dt in range(ND):
            nc.tensor.matmul(
                ps[:],
                lhsT=w_sb[:, dt, :],
                rhs=xt_sb[:, dt, cb * 512:(cb + 1) * 512],
                start=(dt == 0),
                stop=(dt == ND - 1),
            )
        nc.vector.tensor_copy(dst[:, cb * 512:(cb + 1) * 512], ps[:])

    def sc_exp(ci, tt):
        """scores for both heads of key-tile tt -> exp -> pt [128, 2*CH]."""
        sc = sc_ps.tile([128, 2 * CH], f32, tag="sc", name="sc")
        for h in range(2):
            nc.tensor.matmul(
                sc[:, h * CH:(h + 1) * CH],
                lhsT=kT_sb[h * HD:(h + 1) * HD, tt * 128:(tt + 1) * 128],
                rhs=qT_sb[h * HD:(h + 1) * HD, ci * CH:(ci + 1) * CH],
                start=True,
                stop=True,
                tile_position=(h * HD, 0),
            )
        pt = pt_pool.tile([128, 2 * CH], f16, tag="pt", name="pt")
        nc.scalar.activation(pt[:], sc[:], Exp, scale=EXP_SCALE)
        return pt

    def ctx_mm(ctx_ts, tt, pt):
        for h in range(2):
            nc.tensor.matmul(
                ctx_ts[h][:],
                lhsT=vp_sb[:, tt, h * (HD + 1):(h + 1) * (HD + 1)],
                rhs=pt[:, h * CH:(h + 1) * CH],
                start=(tt == 0),
                stop=(tt == NT - 1),
            )

    def norm_a2a(ci, ctx_ts, bc_ps, first=False):
        # denominator row (partition HD of ctx psum) -> sbuf, K=1 matmul
        # broadcast across partitions 0:HD, reciprocal, scale, ship.
        dens = []
        for h in range(2):
            den = den_pool.tile([HD + 1, CH], f16, tag="den", name="den")
            nc.vector.tensor_copy(den[HD:HD + 1, :], ctx_ts[h][HD:HD + 1, :])
            dens.append(den)
        for h in range(2):
            # bc shares the proj PSUM bank (single-buffered): keep the
            # bcast->recip chain strictly per-head so h1's matmul can't
            # clobber h0's bcast before its reciprocal reads it
            bcp = bc_ps.tile([128, CH], f32, tag="pj", name="bcp")
            nc.tensor.matmul(
                bcp[0:HD, :],
                lhsT=ones_sb[HD:HD + 1, :],
                rhs=dens[h][HD:HD + 1, :],
                start=True,
                stop=True,
                tile_position=(HD, 0),
            )
            rb = rbc_pool.tile([HD, CH], f32, tag="rbc", name="rbc")
            nc.vector.reciprocal_approx_fast(rb[:], bcp[0:HD, :])
            cnh = cn_pool.tile([HD, CH], f16, tag=f"cn{h}", name=f"cn{h}")
            nc.vector.tensor_mul(cnh[:], ctx_ts[h][0:HD, :], rb[:])
            if ci <= 1:
                dst = a2a_inA[:, h * HD:(h + 1) * HD,
                              ci * RQ:(ci + 1) * RQ]
            else:
                dst = a2a_inB[ci - 2][:, h * HD:(h + 1) * HD, :]
            nc.sync.dma_start(
                dst.rearrange("r p q -> p r q"),
                cnh[:].rearrange("p (r q) -> p r q", r=NCORES),
            )
        if first:
            # w_out load deferred here so it never contends with x loads
            nc.sync.dma_start(wo_sb[:], wob)

        def a2a(ins, outs):
            nc.gpsimd.collective_compute(
                "AllToAll",
                mybir.AluOpType.bypass,
                replica_groups=[list(range(NCORES))],
                ins=[ins.opt()],
                outs=[outs.opt()],
            )

        if ci == 1:
            # pair A complete: 256KB AllToAll, pull all of ctxf[0]
            a2a(a2a_inA, a2a_outA)
            nc.gpsimd.dma_start(
                ctxf_sb[0][:], a2a_outA.rearrange("r p q -> p r q")
            )
        elif ci >= 2:
            # chunk-sized AllToAll right away: B2 hides under chunk 3's
            # compute, B3 under the tail half-projection
            a2a(a2a_inB[ci - 2], a2a_outB[ci - 2])
            nc.gpsimd.dma_start(
                ctxf_sb[1][:, :, (ci - 2) * RQ:(ci - 1) * RQ],
                a2a_outB[ci - 2].rearrange("r p q -> p r q"),
            )

    def proj_steps(p, state, proj_ps, steps):
        """Emit `steps` increments of proj pair p. 18 steps total: 8 kt
        matmuls + copy for nn=0, then the same for nn=1, then the output
        DMA. One single-bank PSUM accumulator, reused across the halves."""
        for _ in range(steps):
            i = state.get("i", 0)
            if i >= 18:
                return
            state["i"] = i + 1
            if i == 0:
                state["ob"] = ob_pool.tile([128, D], f16, tag="ob", name="ob")
            nn, j = divmod(i, 9)
            if j == 0:
                state["pp"] = proj_ps.tile(
                    [128, 512], f32, tag="pj", name="pj"
                )
            if j < 8:
                nc.tensor.matmul(
                    state["pp"][:],
                    lhsT=ctxf_sb[p][:, j, :],
                    rhs=wo_sb[:, j, nn * 512:(nn + 1) * 512],
                    start=(j == 0),
                    stop=(j == ND - 1),
                )
            else:
                nc.vector.tensor_copy(
                    state["ob"][:, nn * 512:(nn + 1) * 512], state["pp"][:]
                )
                nc.sync.dma_start(
                    out[p][:, nn * 512:(nn + 1) * 512],
                    state["ob"][:, nn * 512:(nn + 1) * 512],
                )

    # ---- phase 1: PE warmup, all qkv chunks + chunk-0 scores, transposes
    pts = {}
    with (
        tc.tile_pool(name="qkv_ps", bufs=1, space="PSUM") as qkv_ps,
        tc.tile_pool(name="tr_ps", bufs=2, space="PSUM") as tr_ps,
    ):
        # ~40 back-to-back identity matmuls flip HAM to 8/8 (~3.4us) while
        # the input DMAs stream in
        warm = qkv_ps.tile([128, 512], f32, tag="qkv0", name="warm")
        for _ in range(120):
            nc.tensor.matmul(
                warm[:, 0:128], lhsT=ident_sb[:, 0:128],
                rhs=ident_sb[:, 0:128], start=True, stop=True,
            )
        qkv_chunk(qT_sb, wq_sb, 0, qkv_ps)
        qkv_chunk(kT_sb, wk_sb, 0, qkv_ps)
        pts[0] = sc_exp(0, 0)
        pts[1] = sc_exp(0, 1)
        qkv_chunk(kT_sb, wk_sb, 1, qkv_ps)
        pts[2] = sc_exp(0, 2)
        pts[3] = sc_exp(0, 3)
        qkv_chunk(kT_sb, wk_sb, 2, qkv_ps)
        pts[4] = sc_exp(0, 4)
        qkv_chunk(kT_sb, wk_sb, 3, qkv_ps)
        pts[5] = sc_exp(0, 5)
        def tr_block(t0, t1):
            for tt in range(t0, t1):
                tp = tr_ps.tile([128, 128], f16, tag="tr", name="tr")
                nc.tensor.transpose(
                    tp[:], vT_sb[:, tt * 128:(tt + 1) * 128], ident_sb[:]
                )
                nc.vector.tensor_copy(vp_sb[:, tt, 0:HD], tp[:, 0:HD])
                nc.vector.tensor_copy(
                    vp_sb[:, tt, HD + 1:2 * HD + 1], tp[:, HD:2 * HD]
                )

        qkv_chunk(vT_sb, wv_sb, 0, qkv_ps)
        pts[6] = sc_exp(0, 6)
        tr_block(0, 4)
        qkv_chunk(vT_sb, wv_sb, 1, qkv_ps)
        pts[7] = sc_exp(0, 7)
        tr_block(4, 8)
        qkv_chunk(vT_sb, wv_sb, 2, qkv_ps)
        pts[8] = sc_exp(0, 8)
        tr_block(8, 12)
        qkv_chunk(vT_sb, wv_sb, 3, qkv_ps)
        pts[9] = sc_exp(0, 9)
        tr_block(12, 16)
        qkv_chunk(qT_sb, wq_sb, 1, qkv_ps)

    # ---- phase 2: psum = sc 4 + ctx 2 + (bc/proj shared) 1 + q23 1 banks --
    with (
        tc.tile_pool(name="ctx_ps", bufs=1, space="PSUM") as ctx_ps,
        tc.tile_pool(name="proj_ps", bufs=1, space="PSUM") as proj_ps,
        tc.tile_pool(name="q23_ps", bufs=1, space="PSUM") as q23_ps,
    ):
        def ctx_tiles():
            return [
                ctx_ps.tile([HD + 1, CH], f32, tag=f"ctx{h}", name=f"ctx{h}")
                for h in range(2)
            ]

        # chunk 0 remaining: sc 10..15 with ctx t=0..11 two-per-slot;
        # every chunk leaves its last ctx t's as leftovers for the next
        # chunk's first slots so the exp stream never waits at boundaries
        cts0 = ctx_tiles()
        for tt in range(10, NT):
            pts[tt] = sc_exp(0, tt)
            for t in (2 * (tt - 10), 2 * (tt - 10) + 1):
                ctx_mm(cts0, t, pts[t])

        prev_cts, prev_pts = cts0, pts
        prev_left = [12, 13, 14, 15]
        pstate = [dict(), dict()]
        for ci in range(1, NCH):
            cts = ctx_tiles()
            cpts = {}
            fillers = []
            if ci in (1, 2):
                # q chunk for the next chunk's queries, one matmul per slot
                cb = ci + 1
                qp = q23_ps.tile([128, 512], f32, tag="q23", name="q23")

                def _qmm(dt, qp=qp, cb=cb):
                    nc.tensor.matmul(
                        qp[:],
                        lhsT=wq_sb[:, dt, :],
                        rhs=xt_sb[:, dt, cb * 512:(cb + 1) * 512],
                        start=(dt == 0),
                        stop=(dt == ND - 1),
                    )

                def _qcopy(qp=qp, cb=cb):
                    nc.vector.tensor_copy(
                        qT_sb[:, cb * 512:(cb + 1) * 512], qp[:]
                    )

                fillers = [lambda dt=dt: _qmm(dt) for dt in range(ND)]
                fillers.append(_qcopy)
            for tt in range(NT):
                cpts[tt] = sc_exp(ci, tt)
                if tt <= 1 and prev_left:
                    k = len(prev_left) // 2
                    for t in (prev_left[:k] if tt == 0 else prev_left[k:]):
                        ctx_mm(prev_cts, t, prev_pts[t])
                if fillers and tt >= 1:
                    fillers.pop(0)()
                if tt == 3:
                    norm_a2a(ci - 1, prev_cts, proj_ps, first=(ci == 1))
                if ci == 3 and 4 <= tt < 13:
                    proj_steps(0, pstate[0], proj_ps, 1)
                if 8 <= tt < 15:
                    for t in (2 * (tt - 8), 2 * (tt - 8) + 1):
                        ctx_mm(cts, t, cpts[t])
            prev_cts, prev_pts = cts, cpts
            prev_left = [14, 15]
        for t in (14, 15):
            ctx_mm(prev_cts, t, prev_pts[t])
        norm_a2a(NCH - 1, prev_cts, proj_ps)
        # tail: pair-1 projection in M=64 halves — the ci2 half's data
        # (B2) landed during chunk 3, so it runs during B3's flight and
        # doubles as the PE keep-alive; the ci3 half follows B3
        proj_steps(0, pstate[0], proj_ps, 18)
        ob1 = ob_pool.tile([128, D], f16, tag="ob", name="ob1")
        for half in range(2):
            for nn in range(2):
                pp = proj_ps.tile([128, 512], f32, tag="pj", name="pjh")
                for kt in range(ND):
                    nc.tensor.matmul(
                        pp[half * HD:(half + 1) * HD, :],
                        lhsT=ctxf_sb[1][:, kt, half * HD:(half + 1) * HD],
                        rhs=wo_sb[:, kt, nn * 512:(nn + 1) * 512],
                        start=(kt == 0),
                        stop=(kt == ND - 1),
                        tile_position=(0, half * HD),
                    )
                nc.vector.tensor_copy(
                    ob1[half * HD:(half + 1) * HD, nn * 512:(nn + 1) * 512],
                    pp[half * HD:(half + 1) * HD, :],
                )
                nc.sync.dma_start(
                    out[1][half * HD:(half + 1) * HD,
                           nn * 512:(nn + 1) * 512],
                    ob1[half * HD:(half + 1) * HD, nn * 512:(nn + 1) * 512],
                )

    es.close()


def get_nc(enable_asserts=False):
    key = ("nc", enable_asserts)
    if key not in _CACHE:
        _CACHE[key] = _build(enable_asserts)
    return _CACHE[key]


def make_in_maps(x, w_in, w_out):
    x = np.asarray(x, dtype=np.float32)
    w_in = np.asarray(w_in, dtype=np.float32)
    w_out = np.asarray(w_out, dtype=np.float32)
    xT = x.T.astype(np.float16)                  # [D, S]
    # [4, 128, ND, 512]: cb-block -> partition -> d-tile -> seq
    xTb = np.ascontiguousarray(
        xT.reshape(ND, 128, 4, 512).transpose(2, 1, 0, 3)
    )
    w_outT = w_out.T.astype(np.float16)          # [A, D]
    wob = np.ascontiguousarray(
        w_outT.reshape(ND, 128, D).transpose(1, 0, 2)
    )

    def wslice(w):                               # [E, D] -> [128, ND, E]
        return np.ascontiguousarray(
            w.T.astype(np.float16).reshape(ND, 128, E).transpose(1, 0, 2)
        )

    in_maps = []
    for c in range(NCORES):
        r0 = c * E
        in_maps.append({
            "xTb": xTb,
            "wqb": wslice(w_in[r0:r0 + E]),
            "wkb": wslice(w_in[A + r0:A + r0 + E]),
            "wvb": wslice(w_in[2 * A + r0:2 * A + r0 + E]),
            "wob": wob,
        })
    return in_maps


def assemble_out(results):
    """results[c]["out"] is [NP, 128, D] fp16; pair p rows 0:64 are global
    rows p*1024 + c*64, rows 64:128 are p*1024 + 512 + c*64."""
    full = np.empty((S, D), dtype=np.float32)
    for c in range(NCORES):
        o = results[c]["out"]
        for p in range(NP):
            r0 = p * 2 * CH + c * RQ
            full[r0:r0 + RQ] = o[p, 0:RQ].astype(np.float32)
            r1 = p * 2 * CH + CH + c * RQ
            full[r1:r1 + RQ] = o[p, RQ:2 * RQ].astype(np.float32)
    return full


def kernel(x, w_in, w_out, tgt_len=None, **kwargs):
    from concourse.bass_utils import run_bass_kernel_spmd

    nc = get_nc()
    in_maps = make_in_maps(x, w_in, w_out)
    res = run_bass_kernel_spmd(nc, in_maps, core_ids=list(range(NCORES)))
    return assemble_out(res.results)
